# revision 1
# baseline (speedup 1.0000x reference)
"""Trainium2 Bass kernel for nn_LocallyConnected3 (B=128, C_in=32, C_out=8, S=8192).

  h[b,j,s]  = tanh(x[b,j,s] * sum_i w1[i,j,s])
  out[b,o,s] = tanh(sum_j h[b,j,s] * w2[o,j,s] + bias[o,s])

Sharding: S axis split across 8 cores (1024 positions each); w2/bias sliced
per core, so no replication of the big tensors.

Per-core layout: SBUF partitions carry (s4, j) with s4 in 0..3 (position
sub-block) and j in 0..31 (in-channel); free dims carry (b, s_in).  Stage 2
is a packed matmul: for each s_in, lhsT = h[(s4,j), b] (stationary),
rhs = block-diag w2 [(s4,j), (o,s4)] built host-side, so one matmul
contracts j for 4 positions at once with k=128.  PSUM comes out as
[b, (o,s4)] — batch on partitions — so stores need no transpose.
"""
import sys

sys.path.insert(0, '/opt/trn_rl_repo')

import numpy as np
import ml_dtypes

import concourse.bass as bass
import concourse.tile as tile
from concourse import mybir
from concourse.alu_op_type import AluOpType
from concourse.bass_utils import run_bass_kernel_spmd

N_CORES = 8
B = 128          # batch
CJ = 32          # C_in
CO = 8           # C_out
S = 8192
SC = S // N_CORES   # 1024 positions per core
ST = 256            # s-tile (4 s4-blocks x SIN s_in)
NT = SC // ST       # 2 s-tiles per core
SIN = 64            # s_in per s-tile
NG = SIN // 4       # psum groups per s-tile
NBC = 8             # b-chunks per s-tile (16 b each)
BC = B // NBC       # 32
F32 = mybir.dt.float32
F16 = mybir.dt.float16


def _patch_tile_drain():
    """core_v3 CTRL instructions accept a single sync-wait; stock
    TileContext packs every final sem wait onto one InstDrain and the pinned
    neuronxcc rejects it.  Spread the waits over single-wait nops."""
    from concourse.tile import ScopedClock, TileContext

    if getattr(TileContext, '_drain_patched', False):
        return

    def _drain_and_barrier_split(self, tick_clock, wait_clock):
        nc = self.nc
        drain_inst = nc.sync.drain()
        wait_clock.add_sem_waits(
            drain_inst.ins, ScopedClock({None: tick_clock.global_clock})
        )
        si = drain_inst.ins.sync_info
        if si is not None and si.on_wait and len(si.on_wait) > 1:
            waits = list(si.on_wait)
            si.on_wait = waits[:1]
            for w in waits[1:]:
                nop = nc.sync.nop(nofuse=True, hint="drain_wait_split")
                nsi = nop.ins.sync_info
                if nsi is None:
                    import bass_rust
                    nop.ins.sync_info = bass_rust.SyncInfo(on_wait=[w], on_update=[])
                else:
                    nsi.on_wait = [w]
        nc.all_engine_barrier()
        assert self.sems is not None
        popped = nc._tile_sem_poison_stack.pop()
        assert popped is self._sem_poison
        nc.clear_and_free_semaphores(list(self.sems.allocated().values()))
        nc.all_engine_barrier()

    TileContext._drain_and_barrier = _drain_and_barrier_split
    TileContext._drain_patched = True


def _build_nc():
    nc = bass.Bass("TRN2")
    # host-packed: [t, p=(s4,j), b, s_in] and [t, p=(s4,j), i, s_in]
    x_d = nc.declare_dram_parameter("xp", [NT, 128, B, SIN], F32, isOutput=False)
    w1_d = nc.declare_dram_parameter("w1p", [NT, 128, CJ, SIN], F32, isOutput=False)
    # host-packed block-diagonal w2: [t, k=(s4,j), s_in, col=(o*4+s4)]
    w2b_d = nc.declare_dram_parameter("w2b", [NT, 128, SIN, 32], F16, isOutput=False)
    # host-packed bias: [(t,g)=64, g4=4, col=(o*4+s4)=32]
    bias_d = nc.declare_dram_parameter("biasb", [NT * NG, 4, 32], F16, isOutput=False)
    out_d = nc.declare_dram_parameter("out", [B, CO, SC], F32, isOutput=True)

    with tile.TileContext(nc) as tc:
        with (
            tc.tile_pool(name="xp", bufs=2) as xp,
            tc.tile_pool(name="hp", bufs=2) as hp,
            tc.tile_pool(name="w1p", bufs=2) as w1p,
            tc.tile_pool(name="w2p", bufs=2) as w2p,
            tc.tile_pool(name="bp", bufs=1) as bp,
            tc.tile_pool(name="sp", bufs=2) as sp,
            tc.tile_pool(name="pp", bufs=8, space="PSUM") as pp,
        ):
            bias_t = bp.tile([1, NT * NG, 4, 32], F16)
            nc.sync.dma_start(bias_t[0:1], bias_d[:].unsqueeze(0))
            ones_t = bp.tile([1, 128], F16)
            nc.vector.memset(ones_t[:], 1.0)

            for t in range(NT):
                s0 = t * ST
                # ---- w1 load [(s4,j), i, s_in] and reduce over i ----
                w1t = w1p.tile([128, CJ, SIN], F32)
                nc.sync.dma_start(w1t[:], w1_d[t])
                for step in (16, 8, 4, 2, 1):
                    nc.vector.tensor_add(
                        w1t[:, 0:step, :], w1t[:, 0:step, :],
                        w1t[:, step:2 * step, :],
                    )
                # ---- stage 1: h = tanh(x * w1s), bf16 ----
                ht = hp.tile([128, B, SIN], F16)
                for bc in range(NBC):
                    xt = xp.tile([128, BC, SIN], F32)
                    nc.sync.dma_start(
                        xt[:], x_d[t, :, bc * BC:(bc + 1) * BC, :]
                    )
                    hsl = ht[:, bc * BC:(bc + 1) * BC, :]
                    nc.vector.tensor_mul(
                        hsl, xt[:],
                        w1t[:, 0:1, :].broadcast_to([128, BC, SIN]),
                    )
                    nc.scalar.activation(
                        hsl, hsl, mybir.ActivationFunctionType.Tanh
                    )
                # ---- stage 2: packed matmuls ----
                w2t = w2p.tile([128, SIN, 32], F16)
                nc.sync.dma_start(w2t[:], w2b_d[t])
                st = sp.tile([128, CO, 4, SIN], F32)
                for g in range(NG):           # groups of 4 s_in
                    ps = pp.tile([128, 4, 32], F32)
                    # bias opener: out[b, col] = bias[col] for every b (k=1)
                    nc.tensor.matmul(
                        ps[:],
                        ones_t[:],
                        bias_t[0:1, t * NG + g],
                        start=True, stop=False,
                    )
                    for g4 in range(4):
                        s_in = g * 4 + g4
                        nc.tensor.matmul(
                            ps[:, g4, :],
                            ht[:, :, s_in],          # lhsT [(s4,j), b]
                            w2t[:, s_in, :],         # rhs  [(s4,j), (o,s4)]
                            start=False, stop=(g4 == 3),
                            skip_group_check=True,
                        )
                    nc.scalar.activation(
                        st[:, :, :, g * 4:(g + 1) * 4].transpose([0, 3, 1, 2]),
                        ps[:],
                        mybir.ActivationFunctionType.Tanh,
                    )
                nc.sync.dma_start(
                    out_d[:, :, s0:s0 + ST]
                    .rearrange("b o (s4 si) -> b o s4 si", s4=4),
                    st[:],
                )
    _split_multi_waits(nc)
    return nc


def _split_multi_waits(nc):
    """core_v3 CTRL sync accepts one wait per instruction (2 for EventSem).
    Hoist excess waits onto same-engine nofuse nops inserted just before."""
    for fn in nc.m.functions:
        for blk in fn.blocks:
            insts = list(blk.instructions)
            if not any(
                i.sync_info is not None and i.sync_info.on_wait
                and len(i.sync_info.on_wait) > 1
                for i in insts
            ):
                continue
            new = []
            for inst in insts:
                si = inst.sync_info
                cap = 2 if isinstance(inst, mybir.InstEventSemaphore) else 1
                if si is not None and si.on_wait and len(si.on_wait) > cap:
                    waits = list(si.on_wait)
                    si.on_wait = waits[:cap]
                    for k, w in enumerate(waits[cap:]):
                        new.append(mybir.InstNoOp(
                            name=f"{inst.name}-ws{k}",
                            engine=inst.engine,
                            bass_nofuse=True,
                            sync_info=mybir.SyncInfo(on_wait=[w], on_update=[]),
                        ))
                new.append(inst)
            try:
                blk.instructions = new
            except AttributeError:
                blk.instructions[:] = new


def _pack_inputs(x, w1, w2, bias):
    """Shard on S and build the per-core packed side tensors."""
    in_maps = []
    for c in range(N_CORES):
        sl = slice(c * SC, (c + 1) * SC)
        # [t, p=(s4,j), b, s_in] partition-major packing
        xc = np.ascontiguousarray(
            x[:, :, sl].reshape(B, CJ, NT, 4, SIN)
            .transpose(2, 3, 1, 0, 4).reshape(NT, 128, B, SIN)
        )
        w1c = np.ascontiguousarray(
            w1[:, :, sl].reshape(CJ, CJ, NT, 4, SIN)
            .transpose(2, 3, 1, 0, 4).reshape(NT, 128, CJ, SIN)
        )
        w2c = w2[:, :, sl]                          # (CO, CJ, SC)
        biasc = bias[:, sl]                         # (CO, SC)

        # block-diag w2: M[t, s4*32+j, s_in, o*4+s4] = w2c[o,j, t*512+s4*128+s_in]
        w2r = w2c.reshape(CO, CJ, NT, 4, SIN)       # o j t s4 s_in
        M = np.zeros((NT, 128, SIN, 32), np.float32)
        for s4 in range(4):
            # [t, j, s_in, o] <- w2r[:, :, :, s4, :]
            M[:, s4 * 32:(s4 + 1) * 32, :, s4::4] = (
                w2r[:, :, :, s4, :].transpose(2, 1, 3, 0)
            )
        w2b = M.astype(np.float16)

        # bias: [(t,g), g4, o*4+s4] = biasc[o, t*512 + s4*128 + g*4+g4]
        br = biasc.reshape(CO, NT, 4, NG, 4)        # o t s4 g g4
        biasb = np.ascontiguousarray(
            br.transpose(1, 3, 4, 0, 2).reshape(NT * NG, 4, 32)
        ).astype(np.float16)
        in_maps.append({"xp": xc, "w1p": w1c, "w2b": w2b, "biasb": biasb})
    return in_maps


_CACHED_NC = None


def kernel(x, w1, w2, bias):
    global _CACHED_NC
    _patch_tile_drain()
    x = np.asarray(x, np.float32)
    w1 = np.asarray(w1, np.float32)
    w2 = np.asarray(w2, np.float32)
    bias = np.asarray(bias, np.float32)

    if _CACHED_NC is None:
        _CACHED_NC = _build_nc()
    nc = _CACHED_NC

    in_maps = _pack_inputs(x, w1, w2, bias)
    res = run_bass_kernel_spmd(nc, in_maps, list(range(N_CORES)))
    out = np.concatenate([res.results[c]["out"] for c in range(N_CORES)], axis=2)
    return out.astype(np.float32)


if __name__ == "__main__":
    rng = np.random.default_rng(0)
    x = rng.standard_normal((B, CJ, S), dtype=np.float32)
    w1 = rng.standard_normal((CJ, CJ, S), dtype=np.float32)
    w2 = rng.standard_normal((CO, CJ, S), dtype=np.float32)
    bias = rng.standard_normal((CO, S), dtype=np.float32)
    out = kernel(x=x, w1=w1, w2=w2, bias=bias)
    h = np.tanh(x * w1.sum(0, keepdims=True))
    ref = np.tanh(np.einsum('bjs,ojs->bos', h, w2) + bias[None])
    err = np.abs(out - ref).max() / max(np.abs(ref).max(), 1e-9)
    print("self-check rel err:", err)



# revision 2
# speedup vs baseline: 1.6568x; 1.6568x over previous
"""Trainium2 Bass kernel for nn_LocallyConnected3 (B=128, C_in=32, C_out=8, S=8192).

  h[b,j,s]  = tanh(x[b,j,s] * sum_i w1[i,j,s])
  out[b,o,s] = tanh(sum_j h[b,j,s] * w2[o,j,s] + bias[o,s])

Sharding: S axis split across 8 cores (1024 positions each).

Per-core layout: SBUF partitions carry (s4, j), s4 in 0..3 (position
sub-block, stride 256) and j in 0..31 (in-channel); positions map to
s_local = s4*256 + u with u in 0..255.  Work is a flat pipeline of 8
chunks of 32 u each; every chunk: DMA x/w1 (f16, packed host-side) ->
vector reduce w1 over i + mul -> scalar tanh -> per 16-u PSUM bank a
bias opener matmul (ones x bias row, N=512) plus 16 packed matmuls
(lhsT = h[(s4,j), b] stationary, rhs = block-diag w2, k=128 contracts
j for 4 positions at once) -> scalar tanh psum->sbuf f16 -> gpsimd
(SWDGE) store.  All engines pipeline chunk-wise; loads ride the SP HW
queue, stores the gpsimd SW queue so neither blocks the other.
"""
import sys

sys.path.insert(0, '/opt/trn_rl_repo')

import numpy as np

import concourse.bass as bass
import concourse.tile as tile
from concourse import mybir
from concourse.bass_utils import run_bass_kernel_spmd

N_CORES = 8
B = 128          # batch
CJ = 32          # C_in
CO = 8           # C_out
S = 8192
SC = S // N_CORES   # 1024 positions per core
NU = SC // 4        # 256 u positions (x4 s4 sub-blocks)
NCH = 8             # x chunks per core
UC = NU // NCH      # 32 u per chunk
NBANK = 16          # psum-bank work units per core
UB = NU // NBANK    # 16 u per bank
F32 = mybir.dt.float32
F16 = mybir.dt.float16


def _patch_tile_drain():
    """core_v3 CTRL instructions accept a single sync-wait; stock
    TileContext packs every final sem wait onto one InstDrain and the pinned
    neuronxcc rejects it.  Spread the waits over single-wait nops."""
    from concourse.tile import ScopedClock, TileContext

    if getattr(TileContext, '_drain_patched', False):
        return

    def _drain_and_barrier_split(self, tick_clock, wait_clock):
        nc = self.nc
        drain_inst = nc.sync.drain()
        wait_clock.add_sem_waits(
            drain_inst.ins, ScopedClock({None: tick_clock.global_clock})
        )
        si = drain_inst.ins.sync_info
        if si is not None and si.on_wait and len(si.on_wait) > 1:
            waits = list(si.on_wait)
            si.on_wait = waits[:1]
            for w in waits[1:]:
                nop = nc.sync.nop(nofuse=True, hint="drain_wait_split")
                nsi = nop.ins.sync_info
                if nsi is None:
                    import bass_rust
                    nop.ins.sync_info = bass_rust.SyncInfo(on_wait=[w], on_update=[])
                else:
                    nsi.on_wait = [w]
        nc.all_engine_barrier()
        assert self.sems is not None
        popped = nc._tile_sem_poison_stack.pop()
        assert popped is self._sem_poison
        nc.clear_and_free_semaphores(list(self.sems.allocated().values()))
        nc.all_engine_barrier()

    TileContext._drain_and_barrier = _drain_and_barrier_split
    TileContext._drain_patched = True


def _build_nc():
    nc = bass.Bass("TRN2")
    # host-packed per core: [ch, p=(s4,j), b, ul] / [ch, p=(s4,j), i, ul]
    x_d = nc.declare_dram_parameter("xp", [NCH, 128, B, UC], F16, isOutput=False)
    w1_d = nc.declare_dram_parameter("w1p", [NCH, 128, CJ, UC], F16, isOutput=False)
    # host-packed block-diag w2: [p=(s4,j), u, col=(o*4+s4)]
    w2_d = nc.declare_dram_parameter("w2b", [128, NU, 32], F16, isOutput=False)
    # host-packed bias: [bank, si, col=(o*4+s4)]
    bias_d = nc.declare_dram_parameter("biasb", [NBANK, UB, 32], F16, isOutput=False)
    # packed output: [bank, b, si, col=(o*4+s4)]; host unpacks
    out_d = nc.declare_dram_parameter("out", [NBANK, B, UB, 32], F16, isOutput=True)

    with tile.TileContext(nc) as tc:
        with (
            tc.tile_pool(name="xp", bufs=3) as xp,
            tc.tile_pool(name="w1p", bufs=2) as w1p,
            tc.tile_pool(name="hp", bufs=3) as hp,
            tc.tile_pool(name="bp", bufs=1) as bp,
            tc.tile_pool(name="sp", bufs=3) as sp,
            tc.tile_pool(name="pp", bufs=6, space="PSUM") as pp,
        ):
            bias_t = bp.tile([1, NBANK, UB, 32], F16)
            nc.sync.dma_start(bias_t[0:1], bias_d[:].unsqueeze(0))
            ones_t = bp.tile([1, 128], F16)
            nc.vector.memset(ones_t[:], 1.0)
            w2t = bp.tile([128, NU, 32], F16)
            nc.sync.dma_start(w2t[:], w2_d[:])

            for ch in range(NCH):
                xt = xp.tile([128, B, UC], F16)
                nc.sync.dma_start(xt[:], x_d[ch])
                w1t = w1p.tile([128, CJ, UC], F16)
                nc.sync.dma_start(w1t[:], w1_d[ch])
                # reduce w1 over i (tree)
                for step in (16, 8, 4, 2, 1):
                    nc.vector.tensor_add(
                        w1t[:, 0:step, :], w1t[:, 0:step, :],
                        w1t[:, step:2 * step, :],
                    )
                # stage 1: h = tanh(x * w1s)
                ht = hp.tile([128, B, UC], F16)
                nc.vector.tensor_mul(
                    ht[:], xt[:],
                    w1t[:, 0:1, :].broadcast_to([128, B, UC]),
                )
                for half in range(2):
                    hsl = ht[:, :, half * UB:(half + 1) * UB]
                    nc.scalar.activation(
                        hsl, hsl, mybir.ActivationFunctionType.Tanh
                    )
                # stage 2: per psum bank, bias opener + 16 packed matmuls
                for half in range(2):
                    bank = ch * 2 + half
                    ps = pp.tile([128, UB, 32], F32)
                    nc.tensor.matmul(
                        ps[:],
                        ones_t[:],
                        bias_t[0:1, bank],
                        start=True, stop=False,
                    )
                    for si in range(UB):
                        u = half * UB + si
                        nc.tensor.matmul(
                            ps[:, si, :],
                            ht[:, :, u],             # lhsT [(s4,j), b]
                            w2t[:, ch * UC + u, :],  # rhs  [(s4,j), (o,s4)]
                            start=False, stop=(si == UB - 1),
                            skip_group_check=True,
                        )
                    st = sp.tile([128, UB, 32], F16)
                    nc.scalar.activation(
                        st[:], ps[:], mybir.ActivationFunctionType.Tanh
                    )
                    nc.gpsimd.dma_start(out_d[bank], st[:])
    _split_multi_waits(nc)
    return nc


def _split_multi_waits(nc):
    """core_v3 CTRL sync accepts one wait per instruction (2 for EventSem).
    Hoist excess waits onto same-engine nofuse nops inserted just before."""
    for fn in nc.m.functions:
        for blk in fn.blocks:
            insts = list(blk.instructions)
            if not any(
                i.sync_info is not None and i.sync_info.on_wait
                and len(i.sync_info.on_wait) > 1
                for i in insts
            ):
                continue
            new = []
            for inst in insts:
                si = inst.sync_info
                cap = 2 if isinstance(inst, mybir.InstEventSemaphore) else 1
                if si is not None and si.on_wait and len(si.on_wait) > cap:
                    waits = list(si.on_wait)
                    si.on_wait = waits[:cap]
                    for k, w in enumerate(waits[cap:]):
                        new.append(mybir.InstNoOp(
                            name=f"{inst.name}-ws{k}",
                            engine=inst.engine,
                            bass_nofuse=True,
                            sync_info=mybir.SyncInfo(on_wait=[w], on_update=[]),
                        ))
                new.append(inst)
            try:
                blk.instructions = new
            except AttributeError:
                blk.instructions[:] = new


def _pack_inputs(x, w1, w2, bias):
    """Shard on S and build the per-core packed f16 side tensors."""
    x = np.asarray(x, np.float32)
    w1 = np.asarray(w1, np.float32)
    w2 = np.asarray(w2, np.float32)
    bias = np.asarray(bias, np.float32)
    in_maps = []
    for c in range(N_CORES):
        sl = slice(c * SC, (c + 1) * SC)
        # x: (b, j, s4, ch, ul) -> [ch, (s4,j), b, ul]
        xc = np.ascontiguousarray(
            x[:, :, sl].reshape(B, CJ, 4, NCH, UC)
            .transpose(3, 2, 1, 0, 4).reshape(NCH, 128, B, UC)
        ).astype(np.float16)
        # w1: (i, j, s4, ch, ul) -> [ch, (s4,j), i, ul]
        w1c = np.ascontiguousarray(
            w1[:, :, sl].reshape(CJ, CJ, 4, NCH, UC)
            .transpose(3, 2, 1, 0, 4).reshape(NCH, 128, CJ, UC)
        ).astype(np.float16)
        # block-diag w2: [(s4,j), u, o*4+s4] = w2[o, j, c*SC + s4*NU + u]
        w2c = w2[:, :, sl].reshape(CO, CJ, 4, NU)        # o j s4 u
        M = np.zeros((4, CJ, NU, 32), np.float32)
        for s4 in range(4):
            M[s4, :, :, s4::4] = w2c[:, :, s4, :].transpose(1, 2, 0)
        w2b = M.reshape(128, NU, 32).astype(np.float16)
        # bias: [bank, si, o*4+s4] = bias[o, c*SC + s4*NU + bank*UB + si]
        bc = bias[:, sl].reshape(CO, 4, NBANK, UB)       # o s4 bank si
        biasb = np.ascontiguousarray(
            bc.transpose(2, 3, 0, 1).reshape(NBANK, UB, 32)
        ).astype(np.float16)
        in_maps.append({"xp": xc, "w1p": w1c, "w2b": w2b, "biasb": biasb})
    return in_maps


_CACHED_NC = None


def kernel(x, w1, w2, bias):
    global _CACHED_NC
    _patch_tile_drain()

    if _CACHED_NC is None:
        _CACHED_NC = _build_nc()
    nc = _CACHED_NC

    in_maps = _pack_inputs(x, w1, w2, bias)
    res = run_bass_kernel_spmd(nc, in_maps, list(range(N_CORES)))
    outs = []
    for c in range(N_CORES):
        o = np.asarray(res.results[c]["out"])            # [bank, b, si, 32]
        oc = (o.reshape(NBANK, B, UB, CO, 4)
              .transpose(1, 3, 4, 0, 2).reshape(B, CO, SC))
        outs.append(oc)
    return np.concatenate(outs, axis=2).astype(np.float32)


if __name__ == "__main__":
    rng = np.random.default_rng(0)
    x = rng.standard_normal((B, CJ, S), dtype=np.float32)
    w1 = rng.standard_normal((CJ, CJ, S), dtype=np.float32)
    w2 = rng.standard_normal((CO, CJ, S), dtype=np.float32)
    bias = rng.standard_normal((CO, S), dtype=np.float32)
    out = kernel(x=x, w1=w1, w2=w2, bias=bias)
    h = np.tanh(x * w1.sum(0, keepdims=True))
    ref = np.tanh(np.einsum('bjs,ojs->bos', h, w2) + bias[None])
    err = np.abs(out - ref).max() / max(np.abs(ref).max(), 1e-9)
    print("self-check rel err:", err)


# revision 7
# speedup vs baseline: 2.1343x; 1.2882x over previous
"""Trainium2 Bass kernel for nn_LocallyConnected3 (B=128, C_in=32, C_out=8, S=8192).

  h[b,j,s]  = tanh(x[b,j,s] * sum_i w1[i,j,s])
  out[b,o,s] = tanh(sum_j h[b,j,s] * w2[o,j,s] + bias[o,s])

Sharding: S axis split across 8 cores (1024 positions each).

Per-core layout: SBUF partitions carry (s4, j), s4 in 0..3 (position
sub-block, stride 256) and j in 0..31 (in-channel); positions map to
s_local = s4*256 + u with u in 0..255.  Work is a flat pipeline of 8
chunks of 32 u each; every chunk: DMA x/w1 (f16, packed host-side) ->
vector reduce w1 over i + mul -> scalar tanh -> per 16-u PSUM bank a
bias opener matmul (ones x bias row, N=512) plus 16 packed matmuls
(lhsT = h[(s4,j), b] stationary, rhs = block-diag w2, k=128 contracts
j for 4 positions at once) -> scalar tanh psum->sbuf f16 -> gpsimd
(SWDGE) store.  All engines pipeline chunk-wise; loads ride the SP HW
queue, stores the gpsimd SW queue so neither blocks the other.
"""
import sys

sys.path.insert(0, '/opt/trn_rl_repo')

import numpy as np

import concourse.bass as bass
import concourse.tile as tile
from concourse import mybir
from concourse.bass_utils import run_bass_kernel_spmd

N_CORES = 8
B = 128          # batch
CJ = 32          # C_in
CO = 8           # C_out
S = 8192
SC = S // N_CORES   # 1024 positions per core
NU = SC // 4        # 256 u positions (x4 s4 sub-blocks)
NCH = 8             # x chunks per core
UC = NU // NCH      # 32 u per chunk
NBANK = 16          # psum-bank work units per core
UB = NU // NBANK    # 16 u per bank
F32 = mybir.dt.float32
F16 = mybir.dt.float16


def _patch_tile_drain():
    """core_v3 CTRL instructions accept a single sync-wait; stock
    TileContext packs every final sem wait onto one InstDrain and the pinned
    neuronxcc rejects it.  Spread the waits over single-wait nops."""
    from concourse.tile import ScopedClock, TileContext

    if getattr(TileContext, '_drain_patched', False):
        return

    def _drain_and_barrier_split(self, tick_clock, wait_clock):
        nc = self.nc
        drain_inst = nc.sync.drain()
        wait_clock.add_sem_waits(
            drain_inst.ins, ScopedClock({None: tick_clock.global_clock})
        )
        si = drain_inst.ins.sync_info
        if si is not None and si.on_wait and len(si.on_wait) > 1:
            waits = list(si.on_wait)
            si.on_wait = waits[:1]
            for w in waits[1:]:
                nop = nc.sync.nop(nofuse=True, hint="drain_wait_split")
                nsi = nop.ins.sync_info
                if nsi is None:
                    import bass_rust
                    nop.ins.sync_info = bass_rust.SyncInfo(on_wait=[w], on_update=[])
                else:
                    nsi.on_wait = [w]
        nc.all_engine_barrier()
        assert self.sems is not None
        popped = nc._tile_sem_poison_stack.pop()
        assert popped is self._sem_poison
        nc.clear_and_free_semaphores(list(self.sems.allocated().values()))
        nc.all_engine_barrier()

    TileContext._drain_and_barrier = _drain_and_barrier_split
    TileContext._drain_patched = True


def _build_nc():
    nc = bass.Bass("TRN2")
    # host-packed per core: [ch, p=(s4,j), b, ul] / [ch, p=(s4,j), i, ul]
    x_d = nc.declare_dram_parameter("xp", [NCH, 128, B, UC], F16, isOutput=False)
    w1_d = nc.declare_dram_parameter("w1p", [NCH, 128, CJ, UC], F16, isOutput=False)
    # host-packed block-diag w2: [ch, p=(s4,j), ul, col=(o*4+s4)]
    w2_d = nc.declare_dram_parameter("w2b", [NCH, 128, UC, 32], F16, isOutput=False)
    # host-packed bias: [bank, si, col=(o*4+s4)]
    bias_d = nc.declare_dram_parameter("biasb", [NBANK, UB, 32], F16, isOutput=False)
    # packed output: [bank, b, si, col=(o*4+s4)]; host unpacks
    out_d = nc.declare_dram_parameter("out", [NBANK, B, UB, 32], F16, isOutput=True)

    with tile.TileContext(nc) as tc:
        with (
            tc.tile_pool(name="xp", bufs=3) as xp,
            tc.tile_pool(name="w1p", bufs=3) as w1p,
            tc.tile_pool(name="w2p", bufs=3) as w2p,
            tc.tile_pool(name="hp", bufs=3) as hp,
            tc.tile_pool(name="bp", bufs=1) as bp,
            tc.tile_pool(name="sp", bufs=3) as sp,
            tc.tile_pool(name="pp", bufs=6, space="PSUM") as pp,
        ):
            bias_t = bp.tile([1, NBANK, UB, 32], F16)
            ones_t = bp.tile([1, 128], F16)
            nc.vector.memset(ones_t[:], 1.0)

            for ch in range(NCH):
                xt = xp.tile([128, B, UC], F16)
                nc.sync.dma_start(xt[:], x_d[ch])
                w1t = w1p.tile([128, CJ, UC], F16)
                nc.sync.dma_start(w1t[:], w1_d[ch])
                w2t = w2p.tile([128, UC, 32], F16)
                nc.sync.dma_start(w2t[:], w2_d[ch])
                if ch == 0:
                    nc.sync.dma_start(bias_t[0:1], bias_d[:].unsqueeze(0))
                # reduce w1 over i (tree)
                for step in (16, 8, 4, 2, 1):
                    nc.vector.tensor_add(
                        w1t[:, 0:step, :], w1t[:, 0:step, :],
                        w1t[:, step:2 * step, :],
                    )
                # stage 1: h = tanh(x * w1s)
                ht = hp.tile([128, B, UC], F16)
                nc.vector.tensor_mul(
                    ht[:], xt[:],
                    w1t[:, 0:1, :].broadcast_to([128, B, UC]),
                )
                nc.scalar.activation(
                    ht[:], ht[:], mybir.ActivationFunctionType.Tanh
                )
                # stage 2: per psum bank, bias opener + 16 packed matmuls
                for half in range(2):
                    bank = ch * 2 + half
                    ps = pp.tile([128, UB, 32], F32)
                    nc.tensor.matmul(
                        ps[:],
                        ones_t[:],
                        bias_t[0:1, bank],
                        start=True, stop=False,
                    )
                    for si in range(UB):
                        u = half * UB + si
                        nc.tensor.matmul(
                            ps[:, si, :],
                            ht[:, :, u],             # lhsT [(s4,j), b]
                            w2t[:, u, :],            # rhs  [(s4,j), (o,s4)]
                            start=False, stop=(si == UB - 1),
                            skip_group_check=True,
                        )
                    st = sp.tile([128, UB, 32], F16)
                    nc.scalar.activation(
                        st[:], ps[:], mybir.ActivationFunctionType.Tanh
                    )
                    nc.gpsimd.dma_start(out_d[bank], st[:])
    _split_multi_waits(nc)
    return nc


def _split_multi_waits(nc):
    """core_v3 CTRL sync accepts one wait per instruction (2 for EventSem).
    Hoist excess waits onto same-engine nofuse nops inserted just before."""
    for fn in nc.m.functions:
        for blk in fn.blocks:
            insts = list(blk.instructions)
            if not any(
                i.sync_info is not None and i.sync_info.on_wait
                and len(i.sync_info.on_wait) > 1
                for i in insts
            ):
                continue
            new = []
            for inst in insts:
                si = inst.sync_info
                cap = 2 if isinstance(inst, mybir.InstEventSemaphore) else 1
                if si is not None and si.on_wait and len(si.on_wait) > cap:
                    waits = list(si.on_wait)
                    si.on_wait = waits[:cap]
                    for k, w in enumerate(waits[cap:]):
                        new.append(mybir.InstNoOp(
                            name=f"{inst.name}-ws{k}",
                            engine=inst.engine,
                            bass_nofuse=True,
                            sync_info=mybir.SyncInfo(on_wait=[w], on_update=[]),
                        ))
                new.append(inst)
            try:
                blk.instructions = new
            except AttributeError:
                blk.instructions[:] = new


def _pack_inputs(x, w1, w2, bias):
    """Shard on S and build the per-core packed f16 side tensors."""
    x = np.asarray(x, np.float32)
    w1 = np.asarray(w1, np.float32)
    w2 = np.asarray(w2, np.float32)
    bias = np.asarray(bias, np.float32)
    in_maps = []
    for c in range(N_CORES):
        sl = slice(c * SC, (c + 1) * SC)
        # x: (b, j, s4, ch, ul) -> [ch, (s4,j), b, ul]
        xc = np.ascontiguousarray(
            x[:, :, sl].reshape(B, CJ, 4, NCH, UC)
            .transpose(3, 2, 1, 0, 4).reshape(NCH, 128, B, UC)
        ).astype(np.float16)
        # w1: (i, j, s4, ch, ul) -> [ch, (s4,j), i, ul]
        w1c = np.ascontiguousarray(
            w1[:, :, sl].reshape(CJ, CJ, 4, NCH, UC)
            .transpose(3, 2, 1, 0, 4).reshape(NCH, 128, CJ, UC)
        ).astype(np.float16)
        # block-diag w2: [ch, (s4,j), ul, o*4+s4] = w2[o, j, c*SC + s4*NU + u]
        w2c = w2[:, :, sl].reshape(CO, CJ, 4, NU)        # o j s4 u
        M = np.zeros((4, CJ, NU, 32), np.float32)
        for s4 in range(4):
            M[s4, :, :, s4::4] = w2c[:, :, s4, :].transpose(1, 2, 0)
        # (s4, j, ch, ul, col) -> [ch, (s4,j), ul, col]
        w2b = np.ascontiguousarray(
            M.reshape(4, CJ, NCH, UC, 32).transpose(2, 0, 1, 3, 4)
            .reshape(NCH, 128, UC, 32)
        ).astype(np.float16)
        # bias: [bank, si, o*4+s4] = bias[o, c*SC + s4*NU + bank*UB + si]
        bc = bias[:, sl].reshape(CO, 4, NBANK, UB)       # o s4 bank si
        biasb = np.ascontiguousarray(
            bc.transpose(2, 3, 0, 1).reshape(NBANK, UB, 32)
        ).astype(np.float16)
        in_maps.append({"xp": xc, "w1p": w1c, "w2b": w2b, "biasb": biasb})
    return in_maps


_CACHED_NC = None


def kernel(x, w1, w2, bias):
    global _CACHED_NC
    _patch_tile_drain()

    if _CACHED_NC is None:
        _CACHED_NC = _build_nc()
    nc = _CACHED_NC

    in_maps = _pack_inputs(x, w1, w2, bias)
    res = run_bass_kernel_spmd(nc, in_maps, list(range(N_CORES)))
    outs = []
    for c in range(N_CORES):
        o = np.asarray(res.results[c]["out"])            # [bank, b, si, 32]
        oc = (o.reshape(NBANK, B, UB, CO, 4)
              .transpose(1, 3, 4, 0, 2).reshape(B, CO, SC))
        outs.append(oc)
    return np.concatenate(outs, axis=2).astype(np.float32)


if __name__ == "__main__":
    rng = np.random.default_rng(0)
    x = rng.standard_normal((B, CJ, S), dtype=np.float32)
    w1 = rng.standard_normal((CJ, CJ, S), dtype=np.float32)
    w2 = rng.standard_normal((CO, CJ, S), dtype=np.float32)
    bias = rng.standard_normal((CO, S), dtype=np.float32)
    out = kernel(x=x, w1=w1, w2=w2, bias=bias)
    h = np.tanh(x * w1.sum(0, keepdims=True))
    ref = np.tanh(np.einsum('bjs,ojs->bos', h, w2) + bias[None])
    err = np.abs(out - ref).max() / max(np.abs(ref).max(), 1e-9)
    print("self-check rel err:", err)
